# revision 31
# baseline (speedup 1.0000x reference)
"""Trainium2 Bass kernel for nn_MemoryLayerCell.

Strategy (data-parallel over batch, 8 cores x 2048 rows):
  * All on-chip activations are FEATURE-major ([feature_partition, batch_free]),
    so every GEMM consumes its producer's layout directly and no on-chip
    transposes are needed. The host pre-transposes inputs and post-transposes
    outputs (numpy).
  * The S=1024 state dimension is permuted on the host ("pi" order: even cells
    first, odd cells second). Under pi:
      - the within-pair partner of feature j is j+-512  -> sigmoid tiles are
        shared/swapped between subtiles f and (f+4)%8, zero data movement;
      - memory_cell_inputs = [+pre, -pre] on contiguous halves -> handled by
        add/sub of the same GEMM2 output, no interleave;
      - cell_out = first 512 features -> GEMM3 reads subtiles 0..3 directly.
  * GEMMs run in bf16 (weights pre-cast on host), fp32 PSUM accumulation.
  * Elementwise chain uses fused scalar_tensor_tensor ops with per-partition
    parameter APs (fully general in the biophysical parameters).
  * LayerNorm: per-batch-column stats via ones-vector matmuls on TensorE,
    mean/rstd broadcast back across partitions with K=1 matmuls.
  * Outputs stored bf16 feature-major; host casts to fp32 and un-permutes.
"""

import numpy as np
import ml_dtypes

B, D, S, O = 16384, 256, 1024, 256
NCORES = 8
BC = B // NCORES            # 2048 batch rows per core
NB = 1024
NMACRO = BC // NB
NH = NB // 512              # 512-wide matmul column groups per macro
EPS = 1e-6
PPW = 18                    # param columns per subtile block

BF16 = ml_dtypes.bfloat16

# pi permutation: new feature j' -> original feature perm[j']
PERM = np.concatenate([np.arange(0, S, 2), np.arange(1, S, 2)])


def _legalize_waits(nc):
    """The installed walrus accepts at most one sync-wait command per
    instruction; Tile emits joins with several. Hoist extra waits onto
    same-engine NoOps inserted right before the instruction."""
    import concourse.mybir as mybir

    for fn in nc.m.functions:
        for blk in fn.blocks:
            out = []
            changed = False
            for ins in blk.instructions:
                si = ins.sync_info
                if si is not None and si.on_wait and len(si.on_wait) > 1:
                    waits = list(si.on_wait)
                    for k, w in enumerate(waits[:-1]):
                        nop = mybir.InstNoOp(name=f"{ins.name}-w{k}")
                        nop.engine = ins.engine
                        nop.sync_info = mybir.SyncInfo(on_wait=[w], on_update=[])
                        out.append(nop)
                    ins.sync_info = mybir.SyncInfo(
                        on_wait=[waits[-1]], on_update=list(si.on_update)
                    )
                    changed = True
                out.append(ins)
            if changed:
                blk.instructions = out


def _build_nc(flags):
    import concourse.bass as bass
    import concourse.mybir as mybir
    from concourse.tile import TileContext

    share_sigma, dual_pre, gamma_trivial, beta_trivial, use_recip_fast = flags
    fp32 = mybir.dt.float32
    bf16 = mybir.dt.bfloat16
    AF = mybir.ActivationFunctionType
    OP = mybir.AluOpType

    nc = bass.Bass(trn_type="TRN2")

    aT = nc.dram_tensor("aT", [D + S, BC], bf16, kind="ExternalInput")
    w12 = nc.dram_tensor("w12", [D + S, S // 2], bf16, kind="ExternalInput")
    w34 = nc.dram_tensor("w34", [S // 2, O], bf16, kind="ExternalInput")
    pp = nc.dram_tensor("pp", [128, 8 * PPW], fp32, kind="ExternalInput")
    zT = nc.dram_tensor("zT", [S + O, BC], bf16, kind="ExternalOutput")

    def ppc(f, k):
        return ppt[:, f * PPW + k : f * PPW + k + 1]

    with TileContext(nc) as tc:
        with (
            tc.tile_pool(name="const", bufs=1) as const,
            tc.tile_pool(name="act", bufs=1) as act,
            tc.tile_pool(name="tmp", bufs=2) as tmp,
            tc.tile_pool(name="small", bufs=1) as small,
            tc.tile_pool(name="psum", bufs=2, space="PSUM") as psum,
        ):
            # ---- constants ----
            ppt = const.tile([128, 8 * PPW], fp32)
            nc.sync.dma_start(out=ppt, in_=pp[:, :])
            w12big = const.tile([128, 10, S // 2], bf16)
            nc.sync.dma_start(out=w12big, in_=w12[:, :].rearrange("(k p) m -> p k m", p=128))
            w12t = [w12big[:, k, :] for k in range(10)]
            w34big = const.tile([128, 4, O], bf16)
            nc.sync.dma_start(out=w34big, in_=w34[:, :].rearrange("(k p) m -> p k m", p=128))
            w34t = [w34big[:, k, :] for k in range(4)]
            ones_col = const.tile([128, 1], bf16)
            nc.vector.memset(ones_col, 1.0 / S)
            ones_row = const.tile([1, 128], fp32)
            nc.vector.memset(ones_row, 1.0)
            eps_ap = const.tile([1, 1], fp32)
            nc.vector.memset(eps_ap, EPS)

            for mi in range(NMACRO):
                c0, c1 = mi * NB, (mi + 1) * NB

                a16big = act.tile([128, 10, NB], bf16, name="a16big", bufs=2)
                aview = aT[:, c0:c1].rearrange("(f p) n -> p f n", p=128)
                nc.sync.dma_start(out=a16big[:, 0:5, :], in_=aview[:, 0:5, :])
                nc.sync.dma_start(out=a16big[:, 5:10, :], in_=aview[:, 5:10, :])
                rhs1 = [a16big[:, k, :] for k in range(10)]
                s16 = rhs1[2:]

                # ---- GEMM12: preT = (W1@W2)^T @ [x; s]  (host-folded) ----
                pre_a, pre_b = [], []
                for m in range(4):
                    pps = psum.tile([128, NB], fp32, name="mmps", tag="mm")
                    for nh in range(NH):
                        sl = slice(512 * nh, 512 * (nh + 1))
                        for k in range(10):
                            nc.tensor.matmul(
                                pps[:, sl],
                                w12t[k][:, 128 * m : 128 * (m + 1)],
                                rhs1[k][:, sl],
                                start=(k == 0),
                                stop=(k == 9),
                            )
                    ta = act.tile([128, NB], bf16, name=f"prea_{m}")
                    nc.scalar.activation(ta, pps, AF.Identity, bias=ppc(m, 12), scale=ppc(m, 11))
                    pre_a.append(ta)
                    if dual_pre:
                        tb = act.tile([128, NB], bf16, name=f"preb_{m}")
                        nc.scalar.activation(tb, pps, AF.Identity, bias=ppc(m, 14), scale=ppc(m, 13))
                        pre_b.append(tb)
                if not dual_pre:
                    pre_b = pre_a

                # ---- sigmoids ----
                sig0 = []
                for f in range(8):
                    t = act.tile([128, NB], bf16, name=f"sig0_{f}")
                    nc.scalar.activation(t, s16[f], AF.Sigmoid, bias=ppc(f, 5), scale=ppc(f, 4))
                    sig0.append(t)
                if share_sigma:
                    sig1 = [sig0[(f + 4) % 8] for f in range(8)]
                else:
                    sig1 = []
                    for f in range(8):
                        t = act.tile([128, NB], bf16, name=f"sig1_{f}")
                        nc.scalar.activation(
                            t, s16[(f + 4) % 8], AF.Sigmoid, bias=ppc(f, 7), scale=ppc(f, 6)
                        )
                        sig1.append(t)

                # ---- elementwise chain -> x16; LN stat accumulation on PE ----
                sxp = psum.tile([1, NB], fp32, name="statps", tag="stats")
                sqp = psum.tile([1, NB], fp32, name="statps", tag="stats")
                x16 = []
                for f in range(8):
                    # w0 = mc0/C*(E-s) = s*(-mc0/C) + mc0*E/C   [TS, 4x mode]
                    at = tmp.tile([128, NB], bf16, name="at", tag="at")
                    nc.gpsimd.tensor_scalar(
                        at, s16[f], ppc(f, 2), ppc(f, 0), OP.mult, OP.add
                    )
                    t0 = tmp.tile([128, NB], bf16, name="t0", tag="t0")
                    nc.vector.tensor_mul(out=t0, in0=at, in1=sig0[f])
                    bt = tmp.tile([128, NB], bf16, name="bt", tag="bt")
                    nc.gpsimd.tensor_scalar(
                        bt, s16[f], ppc(f, 3), ppc(f, 1), OP.mult, OP.add
                    )
                    t1 = tmp.tile([128, NB], bf16, name="t1", tag="t1")
                    nc.vector.tensor_mul(out=t1, in0=bt, in1=sig1[f])
                    u = tmp.tile([128, NB], bf16, name="u", tag="u")
                    nc.vector.tensor_add(out=u, in0=t0, in1=t1)
                    e2 = tmp.tile([128, NB], bf16, name="e2", tag="e2")
                    nc.vector.tensor_add(out=e2, in0=u, in1=s16[f])
                    xt = act.tile([128, NB], bf16, name=f"x16_{f}")
                    if f < 4:
                        nc.vector.tensor_add(out=xt, in0=e2, in1=pre_a[f])
                    else:
                        nc.vector.tensor_sub(out=xt, in0=e2, in1=pre_b[f - 4])
                    x16.append(xt)
                    sq = tmp.tile([128, NB], bf16, name="sq", tag="sq")
                    nc.scalar.activation(sq, xt, AF.Square, bias=0.0, scale=1.0)
                    for nh in range(NH):
                        sl = slice(512 * nh, 512 * (nh + 1))
                        nc.tensor.matmul(
                            sxp[:, sl], ones_col, xt[:, sl],
                            start=(f == 0), stop=(f == 7),
                        )
                        nc.tensor.matmul(
                            sqp[:, sl], ones_col, sq[:, sl],
                            start=(f == 0), stop=(f == 7),
                        )

                # ---- LN stats (sxp = mean, sqp = E[x^2] via 1/S ones) ----
                mu2 = small.tile([1, NB], fp32, name="mu2")
                nc.scalar.activation(mu2, sxp, AF.Square, bias=0.0, scale=1.0)
                var = small.tile([1, NB], fp32, name="var")
                nc.vector.tensor_sub(out=var, in0=sqp, in1=mu2)
                muRow = small.tile([1, NB], fp32, name="muRow")
                nc.scalar.activation(muRow, sxp, AF.Identity, bias=0.0, scale=1.0)
                rstd = small.tile([1, NB], fp32, name="rstd")
                if use_recip_fast:
                    # 1/sqrt(|var + eps|) in one ACT LUT op
                    nc.scalar.activation(
                        rstd, var, AF.Abs_reciprocal_sqrt, bias=eps_ap, scale=1.0
                    )
                else:
                    sd = small.tile([1, NB], fp32, name="sd")
                    nc.scalar.activation(sd, var, AF.Sqrt, bias=eps_ap, scale=1.0)
                    nc.vector.reciprocal(out=rstd, in_=sd)


                # ---- broadcast rows across partitions (K=1 matmuls) ----
                rbp = psum.tile([128, NB], fp32, name="mmps", tag="mm")
                mbp = psum.tile([128, NB], fp32, name="mmps", tag="mm")
                for nh in range(NH):
                    sl = slice(512 * nh, 512 * (nh + 1))
                    nc.tensor.matmul(rbp[:, sl], ones_row, rstd[:, sl], start=True, stop=True)
                    nc.tensor.matmul(mbp[:, sl], ones_row, muRow[:, sl], start=True, stop=True)
                rb16 = small.tile([128, NB], bf16, name="rb16")
                nc.scalar.activation(rb16, rbp, AF.Identity, bias=0.0, scale=1.0)
                mb16 = small.tile([128, NB], bf16, name="mb16")
                nc.scalar.activation(mb16, mbp, AF.Identity, bias=0.0, scale=1.0)

                # ---- normalize: y = (x*rstd - mu*rstd [*gamma]) + beta ----
                yo16big = act.tile([128, 10, NB], bf16, name="yo16big")
                y16 = []
                for f in range(8):
                    zt = tmp.tile([128, NB], bf16, name="zt", tag="zt")
                    nc.vector.tensor_sub(out=zt, in0=x16[f], in1=mb16)
                    yt = yo16big[:, f, :]
                    if gamma_trivial and beta_trivial:
                        nc.vector.tensor_mul(out=yt, in0=zt, in1=rb16)
                    else:
                        z2 = tmp.tile([128, NB], bf16, name="z2", tag="zt")
                        nc.vector.tensor_mul(out=z2, in0=zt, in1=rb16)
                        nc.vector.tensor_scalar(
                            yt, z2, ppc(f, 9), ppc(f, 8), OP.mult, OP.add
                        )
                    y16.append(yt)

                nc.scalar.dma_start(
                    out=zT[:S, c0:c1].rearrange("(f p) n -> p f n", p=128),
                    in_=yo16big[:, 0:8, :],
                )

                # ---- GEMM3': out = W34^T @ y[:512]  (host-folded W3@W4) ----
                for m in range(2):
                    op4 = psum.tile([128, NB], fp32, name="mmps", tag="mm")
                    for nh in range(NH):
                        sl = slice(512 * nh, 512 * (nh + 1))
                        for k in range(4):
                            nc.tensor.matmul(
                                op4[:, sl],
                                w34t[k][:, 128 * m : 128 * (m + 1)],
                                y16[k][:, sl],
                                start=(k == 0),
                                stop=(k == 3),
                            )
                    nc.scalar.activation(
                        yo16big[:, 8 + m, :], op4, AF.Identity, bias=ppc(m, 15), scale=1.0
                    )
                nc.scalar.dma_start(
                    out=zT[S:, c0:c1].rearrange("(f p) n -> p f n", p=128),
                    in_=yo16big[:, 8:10, :],
                )

    _legalize_waits(nc)
    return nc


_CACHE = {}


def _get_nc(flags):
    if flags not in _CACHE:
        _CACHE[flags] = _build_nc(flags)
    return _CACHE[flags]


def kernel(inputs, states, W1, b1, W2, b2, W3, b3, W4, b4,
           excitatory_potential, inhibitory_potential, capacitance,
           max_conductance, mean_conductance_potential, std_conductance,
           gamma, beta, _trace=False):
    from concourse.bass_utils import run_bass_kernel_spmd

    f32 = np.float32
    inputs = np.asarray(inputs, f32)
    states = np.asarray(states, f32)

    # ---- host-side parameter folding (pi space) ----
    C = np.asarray(capacitance, f32)
    mc = np.asarray(max_conductance, f32)
    mean = np.asarray(mean_conductance_potential, f32)
    std = np.asarray(std_conductance, f32)
    E = np.asarray(excitatory_potential, f32)
    I = np.asarray(inhibitory_potential, f32)
    gam = np.asarray(gamma, f32)
    bet = np.asarray(beta, f32)

    Ep, Ip, Cp = E[PERM], I[PERM], C[PERM]
    mc0, mc1 = mc[PERM, 0], mc[PERM, 1]
    a0 = std[PERM, 0]
    cc0 = -std[PERM, 0] * mean[PERM, 0]
    a1 = std[PERM, 1]
    cc1 = -std[PERM, 1] * mean[PERM, 1]
    nmc0 = -mc0 / Cp
    nmc1 = -mc1 / Cp
    gp, bp = gam[PERM], bet[PERM]

    part = (PERM + S // 2) % S  # pi-index of partner's pi-position? see below
    # partner in pi-space of pi-feature j' is j'+-512; its own-params live at
    # pi-position (j'+512)%1024. share if a1/c1 match partner's a0/c0.
    pidx = np.concatenate([np.arange(S // 2, S), np.arange(0, S // 2)])
    share_sigma = bool(
        np.array_equal(a1, a0[pidx]) and np.array_equal(cc1, cc0[pidx])
    )
    C_ev, C_od = Cp[: S // 2], Cp[S // 2 :]
    b2a = np.asarray(b2, f32)
    dual_pre = not (np.array_equal(C_ev, C_od))
    gamma_trivial = bool(np.all(gp == 1.0))
    beta_trivial = bool(np.all(bp == 0.0))

    b12 = np.asarray(b1, f32) @ np.asarray(W2, f32) + b2a
    invCe = 1.0 / C_ev
    invCo = 1.0 / C_od
    b2e = b12 * invCe
    b2o = b12 * invCo

    def cols(v, n=8):
        # [n*128] vector -> per-partition [128, n] columns
        return np.ascontiguousarray(v.reshape(n, 128).T)

    pp = np.zeros((128, 8 * PPW), f32)
    mc0E = mc0 * Ep / Cp
    mc1I = mc1 * Ip / Cp
    for k, v in [(0, mc0E), (1, mc1I), (2, nmc0), (3, nmc1), (4, a0), (5, cc0),
                 (6, a1), (7, cc1), (8, bp), (9, gp), (10, np.asarray(b1, f32))]:
        pp[:, k::PPW] = cols(v)
    for k, v in [(11, invCe), (12, b2e), (13, invCo), (14, b2o)]:
        pp[:, k:k + 4 * PPW:PPW] = cols(v, 4)
    b34 = np.asarray(b3, f32) @ np.asarray(W4, f32) + np.asarray(b4, f32)
    pp[:, 15:15 + 2 * PPW:PPW] = cols(b34, 2)

    # ---- weights (bf16, pi-permuted W1 rows) ----
    W1p = np.asarray(W1, f32).copy()
    W1p[D:, :] = W1p[D + PERM, :]
    w12b = (W1p @ np.asarray(W2, f32)).astype(BF16)
    w34b = (np.asarray(W3, f32) @ np.asarray(W4, f32)).astype(BF16)

    # ---- activations: transpose + permute + bf16 ----
    aTb = np.ascontiguousarray(
        np.concatenate([inputs, states[:, PERM]], axis=1).T.astype(BF16)
    )  # [D+S, B]

    flags = (share_sigma, dual_pre, gamma_trivial, beta_trivial, False)
    nc = _get_nc(flags)

    in_maps = []
    for c in range(NCORES):
        sl = slice(c * BC, (c + 1) * BC)
        in_maps.append({
            "aT": np.ascontiguousarray(aTb[:, sl]),
            "w12": w12b, "w34": w34b, "pp": pp,
        })

    res = run_bass_kernel_spmd(nc, in_maps, core_ids=list(range(NCORES)),
                               trace=_trace)
    kernel._last = res

    zT = np.concatenate([np.asarray(r["zT"]) for r in res.results], axis=1)
    y_pi = zT[:S].astype(f32).T                 # [B, S] in pi order
    next_states = np.empty((B, S), f32)
    next_states[:, PERM] = y_pi
    out = np.ascontiguousarray(zT[S : S + O].astype(f32).T)  # [B, O]
    return (out, next_states)


# revision 33
# speedup vs baseline: 1.0218x; 1.0218x over previous
"""Trainium2 Bass kernel for nn_MemoryLayerCell.

Strategy (data-parallel over batch, 8 cores x 2048 rows):
  * All on-chip activations are FEATURE-major ([feature_partition, batch_free]),
    so every GEMM consumes its producer's layout directly and no on-chip
    transposes are needed. The host pre-transposes inputs and post-transposes
    outputs (numpy).
  * The S=1024 state dimension is permuted on the host ("pi" order: even cells
    first, odd cells second). Under pi:
      - the within-pair partner of feature j is j+-512  -> sigmoid tiles are
        shared/swapped between subtiles f and (f+4)%8, zero data movement;
      - memory_cell_inputs = [+pre, -pre] on contiguous halves -> handled by
        add/sub of the same GEMM2 output, no interleave;
      - cell_out = first 512 features -> GEMM3 reads subtiles 0..3 directly.
  * GEMMs run in bf16 (weights pre-cast on host), fp32 PSUM accumulation.
  * Elementwise chain uses fused scalar_tensor_tensor ops with per-partition
    parameter APs (fully general in the biophysical parameters).
  * LayerNorm: per-batch-column stats via ones-vector matmuls on TensorE,
    mean/rstd broadcast back across partitions with K=1 matmuls.
  * Outputs stored bf16 feature-major; host casts to fp32 and un-permutes.
"""

import numpy as np
import ml_dtypes

B, D, S, O = 16384, 256, 1024, 256
NCORES = 8
BC = B // NCORES            # 2048 batch rows per core
NB = 1024
NMACRO = BC // NB
NH = NB // 512              # 512-wide matmul column groups per macro
EPS = 1e-6
PPW = 18                    # param columns per subtile block

BF16 = ml_dtypes.bfloat16

# pi permutation: new feature j' -> original feature perm[j']
PERM = np.concatenate([np.arange(0, S, 2), np.arange(1, S, 2)])


def _legalize_waits(nc):
    """The installed walrus accepts at most one sync-wait command per
    instruction; Tile emits joins with several. Hoist extra waits onto
    same-engine NoOps inserted right before the instruction."""
    import concourse.mybir as mybir

    for fn in nc.m.functions:
        for blk in fn.blocks:
            out = []
            changed = False
            for ins in blk.instructions:
                si = ins.sync_info
                if si is not None and si.on_wait and len(si.on_wait) > 1:
                    waits = list(si.on_wait)
                    for k, w in enumerate(waits[:-1]):
                        nop = mybir.InstNoOp(name=f"{ins.name}-w{k}")
                        nop.engine = ins.engine
                        nop.sync_info = mybir.SyncInfo(on_wait=[w], on_update=[])
                        out.append(nop)
                    ins.sync_info = mybir.SyncInfo(
                        on_wait=[waits[-1]], on_update=list(si.on_update)
                    )
                    changed = True
                out.append(ins)
            if changed:
                blk.instructions = out


def _build_nc(flags):
    import concourse.bass as bass
    import concourse.mybir as mybir
    from concourse.tile import TileContext

    share_sigma, dual_pre, gamma_trivial, beta_trivial, use_recip_fast = flags
    fp32 = mybir.dt.float32
    bf16 = mybir.dt.bfloat16
    AF = mybir.ActivationFunctionType
    OP = mybir.AluOpType

    nc = bass.Bass(trn_type="TRN2")

    aT = nc.dram_tensor("aT", [D + S, BC], bf16, kind="ExternalInput")
    w12 = nc.dram_tensor("w12", [D + S, S // 2], bf16, kind="ExternalInput")
    w34 = nc.dram_tensor("w34", [S // 2, O], bf16, kind="ExternalInput")
    pp = nc.dram_tensor("pp", [128, 8 * PPW], fp32, kind="ExternalInput")
    zT = nc.dram_tensor("zT", [S + O, BC], bf16, kind="ExternalOutput")

    def ppc(f, k):
        return ppt[:, f * PPW + k : f * PPW + k + 1]

    with TileContext(nc) as tc:
        with (
            tc.tile_pool(name="const", bufs=1) as const,
            tc.tile_pool(name="act", bufs=1) as act,
            tc.tile_pool(name="tmp", bufs=2) as tmp,
            tc.tile_pool(name="small", bufs=1) as small,
            tc.tile_pool(name="psum", bufs=2, space="PSUM") as psum,
        ):
            # ---- constants ----
            ppt = const.tile([128, 8 * PPW], fp32)
            nc.sync.dma_start(out=ppt, in_=pp[:, :])
            w12big = const.tile([128, 10, S // 2], bf16)
            w12v = w12[:, :].rearrange("(k p) m -> p k m", p=128)
            nc.sync.dma_start(out=w12big[:, 0:5, :], in_=w12v[:, 0:5, :])
            nc.sync.dma_start(out=w12big[:, 5:10, :], in_=w12v[:, 5:10, :])
            w12t = [w12big[:, k, :] for k in range(10)]
            w34big = const.tile([128, 4, O], bf16)
            w34t = [w34big[:, k, :] for k in range(4)]
            ones_col = const.tile([128, 1], bf16)
            nc.vector.memset(ones_col, 1.0 / S)
            ones_row = const.tile([1, 128], fp32)
            nc.vector.memset(ones_row, 1.0)
            eps_ap = const.tile([1, 1], fp32)
            nc.vector.memset(eps_ap, EPS)

            for mi in range(NMACRO):
                c0, c1 = mi * NB, (mi + 1) * NB

                a16big = act.tile([128, 10, NB], bf16, name="a16big", bufs=2)
                aview = aT[:, c0:c1].rearrange("(f p) n -> p f n", p=128)
                nc.sync.dma_start(out=a16big[:, 0:5, :], in_=aview[:, 0:5, :])
                nc.sync.dma_start(out=a16big[:, 5:10, :], in_=aview[:, 5:10, :])
                rhs1 = [a16big[:, k, :] for k in range(10)]
                s16 = rhs1[2:]
                if mi == 0:
                    # w34 only feeds the final GEMM; keep it behind the
                    # first input tiles in the load FIFO
                    nc.sync.dma_start(
                        out=w34big, in_=w34[:, :].rearrange("(k p) m -> p k m", p=128)
                    )

                # ---- GEMM12: preT = (W1@W2)^T @ [x; s]  (host-folded) ----
                pre_a, pre_b = [], []
                for m in range(4):
                    pps = psum.tile([128, NB], fp32, name="mmps", tag="mm")
                    for nh in range(NH):
                        sl = slice(512 * nh, 512 * (nh + 1))
                        for k in range(10):
                            nc.tensor.matmul(
                                pps[:, sl],
                                w12t[k][:, 128 * m : 128 * (m + 1)],
                                rhs1[k][:, sl],
                                start=(k == 0),
                                stop=(k == 9),
                            )
                    ta = act.tile([128, NB], bf16, name=f"prea_{m}")
                    nc.scalar.activation(ta, pps, AF.Identity, bias=ppc(m, 12), scale=ppc(m, 11))
                    pre_a.append(ta)
                    if dual_pre:
                        tb = act.tile([128, NB], bf16, name=f"preb_{m}")
                        nc.scalar.activation(tb, pps, AF.Identity, bias=ppc(m, 14), scale=ppc(m, 13))
                        pre_b.append(tb)
                if not dual_pre:
                    pre_b = pre_a

                # ---- sigmoids ----
                sig0 = []
                for f in range(8):
                    t = act.tile([128, NB], bf16, name=f"sig0_{f}")
                    nc.scalar.activation(t, s16[f], AF.Sigmoid, bias=ppc(f, 5), scale=ppc(f, 4))
                    sig0.append(t)
                if share_sigma:
                    sig1 = [sig0[(f + 4) % 8] for f in range(8)]
                else:
                    sig1 = []
                    for f in range(8):
                        t = act.tile([128, NB], bf16, name=f"sig1_{f}")
                        nc.scalar.activation(
                            t, s16[(f + 4) % 8], AF.Sigmoid, bias=ppc(f, 7), scale=ppc(f, 6)
                        )
                        sig1.append(t)

                # ---- elementwise chain -> x16; LN stat accumulation on PE ----
                sxp = psum.tile([1, NB], fp32, name="statps", tag="stats")
                sqp = psum.tile([1, NB], fp32, name="statps", tag="stats")
                x16 = []
                for f in range(8):
                    # w0 = mc0/C*(E-s) = s*(-mc0/C) + mc0*E/C   [TS, 4x mode]
                    at = tmp.tile([128, NB], bf16, name="at", tag="at")
                    nc.gpsimd.tensor_scalar(
                        at, s16[f], ppc(f, 2), ppc(f, 0), OP.mult, OP.add
                    )
                    t0 = tmp.tile([128, NB], bf16, name="t0", tag="t0")
                    nc.vector.tensor_mul(out=t0, in0=at, in1=sig0[f])
                    bt = tmp.tile([128, NB], bf16, name="bt", tag="bt")
                    nc.gpsimd.tensor_scalar(
                        bt, s16[f], ppc(f, 3), ppc(f, 1), OP.mult, OP.add
                    )
                    t1 = tmp.tile([128, NB], bf16, name="t1", tag="t1")
                    nc.vector.tensor_mul(out=t1, in0=bt, in1=sig1[f])
                    u = tmp.tile([128, NB], bf16, name="u", tag="u")
                    nc.vector.tensor_add(out=u, in0=t0, in1=t1)
                    e2 = tmp.tile([128, NB], bf16, name="e2", tag="e2")
                    nc.vector.tensor_add(out=e2, in0=u, in1=s16[f])
                    xt = act.tile([128, NB], bf16, name=f"x16_{f}")
                    if f < 4:
                        nc.vector.tensor_add(out=xt, in0=e2, in1=pre_a[f])
                    else:
                        nc.vector.tensor_sub(out=xt, in0=e2, in1=pre_b[f - 4])
                    x16.append(xt)
                    sq = tmp.tile([128, NB], bf16, name="sq", tag="sq")
                    nc.scalar.activation(sq, xt, AF.Square, bias=0.0, scale=1.0)
                    for nh in range(NH):
                        sl = slice(512 * nh, 512 * (nh + 1))
                        nc.tensor.matmul(
                            sxp[:, sl], ones_col, xt[:, sl],
                            start=(f == 0), stop=(f == 7),
                        )
                        nc.tensor.matmul(
                            sqp[:, sl], ones_col, sq[:, sl],
                            start=(f == 0), stop=(f == 7),
                        )

                # ---- LN stats (sxp = mean, sqp = E[x^2] via 1/S ones) ----
                mu2 = small.tile([1, NB], fp32, name="mu2")
                nc.scalar.activation(mu2, sxp, AF.Square, bias=0.0, scale=1.0)
                var = small.tile([1, NB], fp32, name="var")
                nc.vector.tensor_sub(out=var, in0=sqp, in1=mu2)
                muRow = small.tile([1, NB], fp32, name="muRow")
                nc.scalar.activation(muRow, sxp, AF.Identity, bias=0.0, scale=1.0)
                rstd = small.tile([1, NB], fp32, name="rstd")
                if use_recip_fast:
                    # 1/sqrt(|var + eps|) in one ACT LUT op
                    nc.scalar.activation(
                        rstd, var, AF.Abs_reciprocal_sqrt, bias=eps_ap, scale=1.0
                    )
                else:
                    sd = small.tile([1, NB], fp32, name="sd")
                    nc.scalar.activation(sd, var, AF.Sqrt, bias=eps_ap, scale=1.0)
                    nc.vector.reciprocal(out=rstd, in_=sd)


                # ---- broadcast rows across partitions (K=1 matmuls) ----
                rbp = psum.tile([128, NB], fp32, name="mmps", tag="mm")
                mbp = psum.tile([128, NB], fp32, name="mmps", tag="mm")
                for nh in range(NH):
                    sl = slice(512 * nh, 512 * (nh + 1))
                    nc.tensor.matmul(rbp[:, sl], ones_row, rstd[:, sl], start=True, stop=True)
                    nc.tensor.matmul(mbp[:, sl], ones_row, muRow[:, sl], start=True, stop=True)
                rb16 = small.tile([128, NB], bf16, name="rb16")
                nc.scalar.activation(rb16, rbp, AF.Identity, bias=0.0, scale=1.0)
                mb16 = small.tile([128, NB], bf16, name="mb16")
                nc.scalar.activation(mb16, mbp, AF.Identity, bias=0.0, scale=1.0)

                # ---- normalize: y = (x*rstd - mu*rstd [*gamma]) + beta ----
                yo16big = act.tile([128, 10, NB], bf16, name="yo16big")
                y16 = []
                for f in range(8):
                    zt = tmp.tile([128, NB], bf16, name="zt", tag="zt")
                    nc.vector.tensor_sub(out=zt, in0=x16[f], in1=mb16)
                    yt = yo16big[:, f, :]
                    if gamma_trivial and beta_trivial:
                        nc.vector.tensor_mul(out=yt, in0=zt, in1=rb16)
                    else:
                        z2 = tmp.tile([128, NB], bf16, name="z2", tag="zt")
                        nc.vector.tensor_mul(out=z2, in0=zt, in1=rb16)
                        nc.vector.tensor_scalar(
                            yt, z2, ppc(f, 9), ppc(f, 8), OP.mult, OP.add
                        )
                    y16.append(yt)

                zview = zT[:S, c0:c1].rearrange("(f p) n -> p f n", p=128)
                nc.scalar.dma_start(out=zview[:, 0:4, :], in_=yo16big[:, 0:4, :])
                nc.scalar.dma_start(out=zview[:, 4:8, :], in_=yo16big[:, 4:8, :])

                # ---- GEMM3': out = W34^T @ y[:512]  (host-folded W3@W4) ----
                for m in range(2):
                    op4 = psum.tile([128, NB], fp32, name="mmps", tag="mm")
                    for nh in range(NH):
                        sl = slice(512 * nh, 512 * (nh + 1))
                        for k in range(4):
                            nc.tensor.matmul(
                                op4[:, sl],
                                w34t[k][:, 128 * m : 128 * (m + 1)],
                                y16[k][:, sl],
                                start=(k == 0),
                                stop=(k == 3),
                            )
                    nc.scalar.activation(
                        yo16big[:, 8 + m, :], op4, AF.Identity, bias=ppc(m, 15), scale=1.0
                    )
                nc.scalar.dma_start(
                    out=zT[S:, c0:c1].rearrange("(f p) n -> p f n", p=128),
                    in_=yo16big[:, 8:10, :],
                )

    _legalize_waits(nc)
    return nc


_CACHE = {}


def _get_nc(flags):
    if flags not in _CACHE:
        _CACHE[flags] = _build_nc(flags)
    return _CACHE[flags]


def kernel(inputs, states, W1, b1, W2, b2, W3, b3, W4, b4,
           excitatory_potential, inhibitory_potential, capacitance,
           max_conductance, mean_conductance_potential, std_conductance,
           gamma, beta, _trace=False):
    from concourse.bass_utils import run_bass_kernel_spmd

    f32 = np.float32
    inputs = np.asarray(inputs, f32)
    states = np.asarray(states, f32)

    # ---- host-side parameter folding (pi space) ----
    C = np.asarray(capacitance, f32)
    mc = np.asarray(max_conductance, f32)
    mean = np.asarray(mean_conductance_potential, f32)
    std = np.asarray(std_conductance, f32)
    E = np.asarray(excitatory_potential, f32)
    I = np.asarray(inhibitory_potential, f32)
    gam = np.asarray(gamma, f32)
    bet = np.asarray(beta, f32)

    Ep, Ip, Cp = E[PERM], I[PERM], C[PERM]
    mc0, mc1 = mc[PERM, 0], mc[PERM, 1]
    a0 = std[PERM, 0]
    cc0 = -std[PERM, 0] * mean[PERM, 0]
    a1 = std[PERM, 1]
    cc1 = -std[PERM, 1] * mean[PERM, 1]
    nmc0 = -mc0 / Cp
    nmc1 = -mc1 / Cp
    gp, bp = gam[PERM], bet[PERM]

    part = (PERM + S // 2) % S  # pi-index of partner's pi-position? see below
    # partner in pi-space of pi-feature j' is j'+-512; its own-params live at
    # pi-position (j'+512)%1024. share if a1/c1 match partner's a0/c0.
    pidx = np.concatenate([np.arange(S // 2, S), np.arange(0, S // 2)])
    share_sigma = bool(
        np.array_equal(a1, a0[pidx]) and np.array_equal(cc1, cc0[pidx])
    )
    C_ev, C_od = Cp[: S // 2], Cp[S // 2 :]
    b2a = np.asarray(b2, f32)
    dual_pre = not (np.array_equal(C_ev, C_od))
    gamma_trivial = bool(np.all(gp == 1.0))
    beta_trivial = bool(np.all(bp == 0.0))

    b12 = np.asarray(b1, f32) @ np.asarray(W2, f32) + b2a
    invCe = 1.0 / C_ev
    invCo = 1.0 / C_od
    b2e = b12 * invCe
    b2o = b12 * invCo

    def cols(v, n=8):
        # [n*128] vector -> per-partition [128, n] columns
        return np.ascontiguousarray(v.reshape(n, 128).T)

    pp = np.zeros((128, 8 * PPW), f32)
    mc0E = mc0 * Ep / Cp
    mc1I = mc1 * Ip / Cp
    for k, v in [(0, mc0E), (1, mc1I), (2, nmc0), (3, nmc1), (4, a0), (5, cc0),
                 (6, a1), (7, cc1), (8, bp), (9, gp), (10, np.asarray(b1, f32))]:
        pp[:, k::PPW] = cols(v)
    for k, v in [(11, invCe), (12, b2e), (13, invCo), (14, b2o)]:
        pp[:, k:k + 4 * PPW:PPW] = cols(v, 4)
    b34 = np.asarray(b3, f32) @ np.asarray(W4, f32) + np.asarray(b4, f32)
    pp[:, 15:15 + 2 * PPW:PPW] = cols(b34, 2)

    # ---- weights (bf16, pi-permuted W1 rows) ----
    W1p = np.asarray(W1, f32).copy()
    W1p[D:, :] = W1p[D + PERM, :]
    w12b = (W1p @ np.asarray(W2, f32)).astype(BF16)
    w34b = (np.asarray(W3, f32) @ np.asarray(W4, f32)).astype(BF16)

    # ---- activations: transpose + permute + bf16 ----
    aTb = np.ascontiguousarray(
        np.concatenate([inputs, states[:, PERM]], axis=1).T.astype(BF16)
    )  # [D+S, B]

    flags = (share_sigma, dual_pre, gamma_trivial, beta_trivial, False)
    nc = _get_nc(flags)

    in_maps = []
    for c in range(NCORES):
        sl = slice(c * BC, (c + 1) * BC)
        in_maps.append({
            "aT": np.ascontiguousarray(aTb[:, sl]),
            "w12": w12b, "w34": w34b, "pp": pp,
        })

    res = run_bass_kernel_spmd(nc, in_maps, core_ids=list(range(NCORES)),
                               trace=_trace)
    kernel._last = res

    zT = np.concatenate([np.asarray(r["zT"]) for r in res.results], axis=1)
    y_pi = zT[:S].astype(f32).T                 # [B, S] in pi order
    next_states = np.empty((B, S), f32)
    next_states[:, PERM] = y_pi
    out = np.ascontiguousarray(zT[S : S + O].astype(f32).T)  # [B, O]
    return (out, next_states)


# revision 34
# speedup vs baseline: 1.0510x; 1.0285x over previous
"""Trainium2 Bass kernel for nn_MemoryLayerCell.

Strategy (data-parallel over batch, 8 cores x 2048 rows):
  * All on-chip activations are FEATURE-major ([feature_partition, batch_free]),
    so every GEMM consumes its producer's layout directly and no on-chip
    transposes are needed. The host pre-transposes inputs and post-transposes
    outputs (numpy).
  * The S=1024 state dimension is permuted on the host ("pi" order: even cells
    first, odd cells second). Under pi:
      - the within-pair partner of feature j is j+-512  -> sigmoid tiles are
        shared/swapped between subtiles f and (f+4)%8, zero data movement;
      - memory_cell_inputs = [+pre, -pre] on contiguous halves -> handled by
        add/sub of the same GEMM2 output, no interleave;
      - cell_out = first 512 features -> GEMM3 reads subtiles 0..3 directly.
  * GEMMs run in bf16 (weights pre-cast on host), fp32 PSUM accumulation.
  * Elementwise chain uses fused scalar_tensor_tensor ops with per-partition
    parameter APs (fully general in the biophysical parameters).
  * LayerNorm: per-batch-column stats via ones-vector matmuls on TensorE,
    mean/rstd broadcast back across partitions with K=1 matmuls.
  * Outputs stored bf16 feature-major; host casts to fp32 and un-permutes.
"""

import numpy as np
import ml_dtypes

B, D, S, O = 16384, 256, 1024, 256
NCORES = 8
BC = B // NCORES            # 2048 batch rows per core
NB = 1024
NMACRO = BC // NB
NH = NB // 512              # 512-wide matmul column groups per macro
EPS = 1e-6
PPW = 18                    # param columns per subtile block

BF16 = ml_dtypes.bfloat16

# pi permutation: new feature j' -> original feature perm[j']
PERM = np.concatenate([np.arange(0, S, 2), np.arange(1, S, 2)])


def _legalize_waits(nc):
    """The installed walrus accepts at most one sync-wait command per
    instruction; Tile emits joins with several. Hoist extra waits onto
    same-engine NoOps inserted right before the instruction."""
    import concourse.mybir as mybir

    for fn in nc.m.functions:
        for blk in fn.blocks:
            out = []
            changed = False
            for ins in blk.instructions:
                si = ins.sync_info
                if si is not None and si.on_wait and len(si.on_wait) > 1:
                    waits = list(si.on_wait)
                    for k, w in enumerate(waits[:-1]):
                        nop = mybir.InstNoOp(name=f"{ins.name}-w{k}")
                        nop.engine = ins.engine
                        nop.sync_info = mybir.SyncInfo(on_wait=[w], on_update=[])
                        out.append(nop)
                    ins.sync_info = mybir.SyncInfo(
                        on_wait=[waits[-1]], on_update=list(si.on_update)
                    )
                    changed = True
                out.append(ins)
            if changed:
                blk.instructions = out


def _build_nc(flags):
    import concourse.bass as bass
    import concourse.mybir as mybir
    from concourse.tile import TileContext

    share_sigma, dual_pre, gamma_trivial, beta_trivial, use_recip_fast = flags
    fp32 = mybir.dt.float32
    bf16 = mybir.dt.bfloat16
    AF = mybir.ActivationFunctionType
    OP = mybir.AluOpType

    nc = bass.Bass(trn_type="TRN2")

    aT = nc.dram_tensor("aT", [D + S, BC], bf16, kind="ExternalInput")
    w12 = nc.dram_tensor("w12", [D + S, S // 2], bf16, kind="ExternalInput")
    w34 = nc.dram_tensor("w34", [S // 2, O], bf16, kind="ExternalInput")
    pp = nc.dram_tensor("pp", [128, 8 * PPW], fp32, kind="ExternalInput")
    zT = nc.dram_tensor("zT", [S + O, BC], bf16, kind="ExternalOutput")

    def ppc(f, k):
        return ppt[:, f * PPW + k : f * PPW + k + 1]

    with TileContext(nc) as tc:
        with (
            tc.tile_pool(name="const", bufs=1) as const,
            tc.tile_pool(name="act", bufs=1) as act,
            tc.tile_pool(name="tmp", bufs=2) as tmp,
            tc.tile_pool(name="small", bufs=1) as small,
            tc.tile_pool(name="psum", bufs=2, space="PSUM") as psum,
        ):
            # ---- constants ----
            ppt = const.tile([128, 8 * PPW], fp32)
            nc.sync.dma_start(out=ppt, in_=pp[:, :])
            w12big = const.tile([128, 10, S // 2], bf16)
            w12v = w12[:, :].rearrange("(k p) m -> p k m", p=128)
            nc.sync.dma_start(out=w12big[:, 0:5, :], in_=w12v[:, 0:5, :])
            w12t = [w12big[:, k, :] for k in range(10)]
            w34big = const.tile([128, 4, O], bf16)
            w34t = [w34big[:, k, :] for k in range(4)]
            ones_col = const.tile([128, 1], bf16)
            nc.vector.memset(ones_col, 1.0 / S)
            ones_row = const.tile([1, 128], fp32)
            nc.vector.memset(ones_row, 1.0)
            eps_ap = const.tile([1, 1], fp32)
            nc.vector.memset(eps_ap, EPS)

            for mi in range(NMACRO):
                c0, c1 = mi * NB, (mi + 1) * NB

                a16big = act.tile([128, 10, NB], bf16, name="a16big", bufs=2)
                aview = aT[:, c0:c1].rearrange("(f p) n -> p f n", p=128)
                nc.sync.dma_start(out=a16big[:, 0:5, :], in_=aview[:, 0:5, :])
                if mi == 0:
                    nc.sync.dma_start(out=w12big[:, 5:10, :], in_=w12v[:, 5:10, :])
                nc.sync.dma_start(out=a16big[:, 5:10, :], in_=aview[:, 5:10, :])
                rhs1 = [a16big[:, k, :] for k in range(10)]
                s16 = rhs1[2:]
                if mi == 0:
                    # w34 only feeds the final GEMM; keep it behind the
                    # first input tiles in the load FIFO
                    nc.sync.dma_start(
                        out=w34big, in_=w34[:, :].rearrange("(k p) m -> p k m", p=128)
                    )

                # ---- GEMM12: preT = (W1@W2)^T @ [x; s]  (host-folded) ----
                pre_a, pre_b = [], []
                for m in range(4):
                    pps = psum.tile([128, NB], fp32, name="mmps", tag="mm")
                    for nh in range(NH):
                        sl = slice(512 * nh, 512 * (nh + 1))
                        for k in range(10):
                            nc.tensor.matmul(
                                pps[:, sl],
                                w12t[k][:, 128 * m : 128 * (m + 1)],
                                rhs1[k][:, sl],
                                start=(k == 0),
                                stop=(k == 9),
                            )
                    ta = act.tile([128, NB], bf16, name=f"prea_{m}")
                    nc.scalar.activation(ta, pps, AF.Identity, bias=ppc(m, 12), scale=ppc(m, 11))
                    pre_a.append(ta)
                    if dual_pre:
                        tb = act.tile([128, NB], bf16, name=f"preb_{m}")
                        nc.scalar.activation(tb, pps, AF.Identity, bias=ppc(m, 14), scale=ppc(m, 13))
                        pre_b.append(tb)
                if not dual_pre:
                    pre_b = pre_a

                # ---- sigmoids ----
                sig0 = []
                for f in range(8):
                    t = act.tile([128, NB], bf16, name=f"sig0_{f}")
                    nc.scalar.activation(t, s16[f], AF.Sigmoid, bias=ppc(f, 5), scale=ppc(f, 4))
                    sig0.append(t)
                if share_sigma:
                    sig1 = [sig0[(f + 4) % 8] for f in range(8)]
                else:
                    sig1 = []
                    for f in range(8):
                        t = act.tile([128, NB], bf16, name=f"sig1_{f}")
                        nc.scalar.activation(
                            t, s16[(f + 4) % 8], AF.Sigmoid, bias=ppc(f, 7), scale=ppc(f, 6)
                        )
                        sig1.append(t)

                # ---- elementwise chain -> x16; LN stat accumulation on PE ----
                sxp = psum.tile([1, NB], fp32, name="statps", tag="stats")
                sqp = psum.tile([1, NB], fp32, name="statps", tag="stats")
                x16 = []
                for f in range(8):
                    # w0 = mc0/C*(E-s) = s*(-mc0/C) + mc0*E/C   [TS, 4x mode]
                    at = tmp.tile([128, NB], bf16, name="at", tag="at")
                    nc.gpsimd.tensor_scalar(
                        at, s16[f], ppc(f, 2), ppc(f, 0), OP.mult, OP.add
                    )
                    t0 = tmp.tile([128, NB], bf16, name="t0", tag="t0")
                    nc.vector.tensor_mul(out=t0, in0=at, in1=sig0[f])
                    bt = tmp.tile([128, NB], bf16, name="bt", tag="bt")
                    nc.gpsimd.tensor_scalar(
                        bt, s16[f], ppc(f, 3), ppc(f, 1), OP.mult, OP.add
                    )
                    t1 = tmp.tile([128, NB], bf16, name="t1", tag="t1")
                    nc.vector.tensor_mul(out=t1, in0=bt, in1=sig1[f])
                    u = tmp.tile([128, NB], bf16, name="u", tag="u")
                    nc.vector.tensor_add(out=u, in0=t0, in1=t1)
                    e2 = tmp.tile([128, NB], bf16, name="e2", tag="e2")
                    nc.vector.tensor_add(out=e2, in0=u, in1=s16[f])
                    xt = act.tile([128, NB], bf16, name=f"x16_{f}")
                    if f < 4:
                        nc.vector.tensor_add(out=xt, in0=e2, in1=pre_a[f])
                    else:
                        nc.vector.tensor_sub(out=xt, in0=e2, in1=pre_b[f - 4])
                    x16.append(xt)
                    sq = tmp.tile([128, NB], bf16, name="sq", tag="sq")
                    nc.scalar.activation(sq, xt, AF.Square, bias=0.0, scale=1.0)
                    for nh in range(NH):
                        sl = slice(512 * nh, 512 * (nh + 1))
                        nc.tensor.matmul(
                            sxp[:, sl], ones_col, xt[:, sl],
                            start=(f == 0), stop=(f == 7),
                        )
                        nc.tensor.matmul(
                            sqp[:, sl], ones_col, sq[:, sl],
                            start=(f == 0), stop=(f == 7),
                        )

                # ---- LN stats (sxp = mean, sqp = E[x^2] via 1/S ones) ----
                mu2 = small.tile([1, NB], fp32, name="mu2")
                nc.scalar.activation(mu2, sxp, AF.Square, bias=0.0, scale=1.0)
                var = small.tile([1, NB], fp32, name="var")
                nc.vector.tensor_sub(out=var, in0=sqp, in1=mu2)
                muRow = small.tile([1, NB], fp32, name="muRow")
                nc.scalar.activation(muRow, sxp, AF.Identity, bias=0.0, scale=1.0)
                rstd = small.tile([1, NB], fp32, name="rstd")
                if use_recip_fast:
                    # 1/sqrt(|var + eps|) in one ACT LUT op
                    nc.scalar.activation(
                        rstd, var, AF.Abs_reciprocal_sqrt, bias=eps_ap, scale=1.0
                    )
                else:
                    sd = small.tile([1, NB], fp32, name="sd")
                    nc.scalar.activation(sd, var, AF.Sqrt, bias=eps_ap, scale=1.0)
                    nc.vector.reciprocal(out=rstd, in_=sd)


                # ---- broadcast rows across partitions (K=1 matmuls) ----
                rbp = psum.tile([128, NB], fp32, name="mmps", tag="mm")
                mbp = psum.tile([128, NB], fp32, name="mmps", tag="mm")
                for nh in range(NH):
                    sl = slice(512 * nh, 512 * (nh + 1))
                    nc.tensor.matmul(rbp[:, sl], ones_row, rstd[:, sl], start=True, stop=True)
                    nc.tensor.matmul(mbp[:, sl], ones_row, muRow[:, sl], start=True, stop=True)
                rb16 = small.tile([128, NB], bf16, name="rb16")
                nc.scalar.activation(rb16, rbp, AF.Identity, bias=0.0, scale=1.0)
                mb16 = small.tile([128, NB], bf16, name="mb16")
                nc.scalar.activation(mb16, mbp, AF.Identity, bias=0.0, scale=1.0)

                # ---- normalize: y = (x*rstd - mu*rstd [*gamma]) + beta ----
                yo16big = act.tile([128, 10, NB], bf16, name="yo16big")
                y16 = []
                for f in range(8):
                    zt = tmp.tile([128, NB], bf16, name="zt", tag="zt")
                    nc.vector.tensor_sub(out=zt, in0=x16[f], in1=mb16)
                    yt = yo16big[:, f, :]
                    if gamma_trivial and beta_trivial:
                        nc.vector.tensor_mul(out=yt, in0=zt, in1=rb16)
                    else:
                        z2 = tmp.tile([128, NB], bf16, name="z2", tag="zt")
                        nc.vector.tensor_mul(out=z2, in0=zt, in1=rb16)
                        nc.vector.tensor_scalar(
                            yt, z2, ppc(f, 9), ppc(f, 8), OP.mult, OP.add
                        )
                    y16.append(yt)

                zview = zT[:S, c0:c1].rearrange("(f p) n -> p f n", p=128)
                nc.scalar.dma_start(out=zview[:, 0:4, :], in_=yo16big[:, 0:4, :])
                nc.scalar.dma_start(out=zview[:, 4:8, :], in_=yo16big[:, 4:8, :])

                # ---- GEMM3': out = W34^T @ y[:512]  (host-folded W3@W4) ----
                for m in range(2):
                    op4 = psum.tile([128, NB], fp32, name="mmps", tag="mm")
                    for nh in range(NH):
                        sl = slice(512 * nh, 512 * (nh + 1))
                        for k in range(4):
                            nc.tensor.matmul(
                                op4[:, sl],
                                w34t[k][:, 128 * m : 128 * (m + 1)],
                                y16[k][:, sl],
                                start=(k == 0),
                                stop=(k == 3),
                            )
                    nc.scalar.activation(
                        yo16big[:, 8 + m, :], op4, AF.Identity, bias=ppc(m, 15), scale=1.0
                    )
                nc.scalar.dma_start(
                    out=zT[S:, c0:c1].rearrange("(f p) n -> p f n", p=128),
                    in_=yo16big[:, 8:10, :],
                )

    _legalize_waits(nc)
    return nc


_CACHE = {}


def _get_nc(flags):
    if flags not in _CACHE:
        _CACHE[flags] = _build_nc(flags)
    return _CACHE[flags]


def kernel(inputs, states, W1, b1, W2, b2, W3, b3, W4, b4,
           excitatory_potential, inhibitory_potential, capacitance,
           max_conductance, mean_conductance_potential, std_conductance,
           gamma, beta, _trace=False):
    from concourse.bass_utils import run_bass_kernel_spmd

    f32 = np.float32
    inputs = np.asarray(inputs, f32)
    states = np.asarray(states, f32)

    # ---- host-side parameter folding (pi space) ----
    C = np.asarray(capacitance, f32)
    mc = np.asarray(max_conductance, f32)
    mean = np.asarray(mean_conductance_potential, f32)
    std = np.asarray(std_conductance, f32)
    E = np.asarray(excitatory_potential, f32)
    I = np.asarray(inhibitory_potential, f32)
    gam = np.asarray(gamma, f32)
    bet = np.asarray(beta, f32)

    Ep, Ip, Cp = E[PERM], I[PERM], C[PERM]
    mc0, mc1 = mc[PERM, 0], mc[PERM, 1]
    a0 = std[PERM, 0]
    cc0 = -std[PERM, 0] * mean[PERM, 0]
    a1 = std[PERM, 1]
    cc1 = -std[PERM, 1] * mean[PERM, 1]
    nmc0 = -mc0 / Cp
    nmc1 = -mc1 / Cp
    gp, bp = gam[PERM], bet[PERM]

    part = (PERM + S // 2) % S  # pi-index of partner's pi-position? see below
    # partner in pi-space of pi-feature j' is j'+-512; its own-params live at
    # pi-position (j'+512)%1024. share if a1/c1 match partner's a0/c0.
    pidx = np.concatenate([np.arange(S // 2, S), np.arange(0, S // 2)])
    share_sigma = bool(
        np.array_equal(a1, a0[pidx]) and np.array_equal(cc1, cc0[pidx])
    )
    C_ev, C_od = Cp[: S // 2], Cp[S // 2 :]
    b2a = np.asarray(b2, f32)
    dual_pre = not (np.array_equal(C_ev, C_od))
    gamma_trivial = bool(np.all(gp == 1.0))
    beta_trivial = bool(np.all(bp == 0.0))

    b12 = np.asarray(b1, f32) @ np.asarray(W2, f32) + b2a
    invCe = 1.0 / C_ev
    invCo = 1.0 / C_od
    b2e = b12 * invCe
    b2o = b12 * invCo

    def cols(v, n=8):
        # [n*128] vector -> per-partition [128, n] columns
        return np.ascontiguousarray(v.reshape(n, 128).T)

    pp = np.zeros((128, 8 * PPW), f32)
    mc0E = mc0 * Ep / Cp
    mc1I = mc1 * Ip / Cp
    for k, v in [(0, mc0E), (1, mc1I), (2, nmc0), (3, nmc1), (4, a0), (5, cc0),
                 (6, a1), (7, cc1), (8, bp), (9, gp), (10, np.asarray(b1, f32))]:
        pp[:, k::PPW] = cols(v)
    for k, v in [(11, invCe), (12, b2e), (13, invCo), (14, b2o)]:
        pp[:, k:k + 4 * PPW:PPW] = cols(v, 4)
    b34 = np.asarray(b3, f32) @ np.asarray(W4, f32) + np.asarray(b4, f32)
    pp[:, 15:15 + 2 * PPW:PPW] = cols(b34, 2)

    # ---- weights (bf16, pi-permuted W1 rows) ----
    W1p = np.asarray(W1, f32).copy()
    W1p[D:, :] = W1p[D + PERM, :]
    w12b = (W1p @ np.asarray(W2, f32)).astype(BF16)
    w34b = (np.asarray(W3, f32) @ np.asarray(W4, f32)).astype(BF16)

    # ---- activations: transpose + permute + bf16 ----
    aTb = np.ascontiguousarray(
        np.concatenate([inputs, states[:, PERM]], axis=1).T.astype(BF16)
    )  # [D+S, B]

    flags = (share_sigma, dual_pre, gamma_trivial, beta_trivial, False)
    nc = _get_nc(flags)

    in_maps = []
    for c in range(NCORES):
        sl = slice(c * BC, (c + 1) * BC)
        in_maps.append({
            "aT": np.ascontiguousarray(aTb[:, sl]),
            "w12": w12b, "w34": w34b, "pp": pp,
        })

    res = run_bass_kernel_spmd(nc, in_maps, core_ids=list(range(NCORES)),
                               trace=_trace)
    kernel._last = res

    zT = np.concatenate([np.asarray(r["zT"]) for r in res.results], axis=1)
    y_pi = zT[:S].astype(f32).T                 # [B, S] in pi order
    next_states = np.empty((B, S), f32)
    next_states[:, PERM] = y_pi
    out = np.ascontiguousarray(zT[S : S + O].astype(f32).T)  # [B, O]
    return (out, next_states)


# revision 35
# speedup vs baseline: 1.0670x; 1.0153x over previous
"""Trainium2 Bass kernel for nn_MemoryLayerCell.

Strategy (data-parallel over batch, 8 cores x 2048 rows):
  * All on-chip activations are FEATURE-major ([feature_partition, batch_free]),
    so every GEMM consumes its producer's layout directly and no on-chip
    transposes are needed. The host pre-transposes inputs and post-transposes
    outputs (numpy).
  * The S=1024 state dimension is permuted on the host ("pi" order: even cells
    first, odd cells second). Under pi:
      - the within-pair partner of feature j is j+-512  -> sigmoid tiles are
        shared/swapped between subtiles f and (f+4)%8, zero data movement;
      - memory_cell_inputs = [+pre, -pre] on contiguous halves -> handled by
        add/sub of the same GEMM2 output, no interleave;
      - cell_out = first 512 features -> GEMM3 reads subtiles 0..3 directly.
  * GEMMs run in bf16 (weights pre-cast on host), fp32 PSUM accumulation.
  * Elementwise chain uses fused scalar_tensor_tensor ops with per-partition
    parameter APs (fully general in the biophysical parameters).
  * LayerNorm: per-batch-column stats via ones-vector matmuls on TensorE,
    mean/rstd broadcast back across partitions with K=1 matmuls.
  * Outputs stored bf16 feature-major; host casts to fp32 and un-permutes.
"""

import numpy as np
import ml_dtypes

B, D, S, O = 16384, 256, 1024, 256
NCORES = 8
BC = B // NCORES            # 2048 batch rows per core
NB = 1024
NMACRO = BC // NB
NH = NB // 512              # 512-wide matmul column groups per macro
EPS = 1e-6
PPW = 18                    # param columns per subtile block

BF16 = ml_dtypes.bfloat16

# pi permutation: new feature j' -> original feature perm[j']
PERM = np.concatenate([np.arange(0, S, 2), np.arange(1, S, 2)])


def _legalize_waits(nc):
    """The installed walrus accepts at most one sync-wait command per
    instruction; Tile emits joins with several. Hoist extra waits onto
    same-engine NoOps inserted right before the instruction."""
    import concourse.mybir as mybir

    for fn in nc.m.functions:
        for blk in fn.blocks:
            out = []
            changed = False
            for ins in blk.instructions:
                si = ins.sync_info
                if si is not None and si.on_wait and len(si.on_wait) > 1:
                    waits = list(si.on_wait)
                    for k, w in enumerate(waits[:-1]):
                        nop = mybir.InstNoOp(name=f"{ins.name}-w{k}")
                        nop.engine = ins.engine
                        nop.sync_info = mybir.SyncInfo(on_wait=[w], on_update=[])
                        out.append(nop)
                    ins.sync_info = mybir.SyncInfo(
                        on_wait=[waits[-1]], on_update=list(si.on_update)
                    )
                    changed = True
                out.append(ins)
            if changed:
                blk.instructions = out


def _build_nc(flags):
    import concourse.bass as bass
    import concourse.mybir as mybir
    from concourse.tile import TileContext

    share_sigma, dual_pre, gamma_trivial, beta_trivial, use_recip_fast = flags
    fp32 = mybir.dt.float32
    bf16 = mybir.dt.bfloat16
    AF = mybir.ActivationFunctionType
    OP = mybir.AluOpType

    nc = bass.Bass(trn_type="TRN2")

    aT = nc.dram_tensor("aT", [D + S, BC], bf16, kind="ExternalInput")
    w12 = nc.dram_tensor("w12", [D + S, S // 2], bf16, kind="ExternalInput")
    w34 = nc.dram_tensor("w34", [S // 2, O], bf16, kind="ExternalInput")
    pp = nc.dram_tensor("pp", [128, 8 * PPW], fp32, kind="ExternalInput")
    zT = nc.dram_tensor("zT", [S + O, BC], bf16, kind="ExternalOutput")

    def ppc(f, k):
        return ppt[:, f * PPW + k : f * PPW + k + 1]

    with TileContext(nc) as tc:
        with (
            tc.tile_pool(name="const", bufs=1) as const,
            tc.tile_pool(name="act", bufs=1) as act,
            tc.tile_pool(name="tmp", bufs=2) as tmp,
            tc.tile_pool(name="small", bufs=1) as small,
            tc.tile_pool(name="psum", bufs=2, space="PSUM") as psum,
        ):
            # ---- constants ----
            ppt = const.tile([128, 8 * PPW], fp32)
            nc.sync.dma_start(out=ppt, in_=pp[:, :])
            w12big = const.tile([128, 10, S // 2], bf16)
            w12v = w12[:, :].rearrange("(k p) m -> p k m", p=128)
            nc.sync.dma_start(out=w12big[:, 0:5, :], in_=w12v[:, 0:5, :])
            w12t = [w12big[:, k, :] for k in range(10)]
            w34big = const.tile([128, 4, O], bf16)
            w34t = [w34big[:, k, :] for k in range(4)]
            ones_col = const.tile([128, 1], bf16)
            nc.vector.memset(ones_col, 1.0 / S)
            ones_row = const.tile([1, 128], fp32)
            nc.vector.memset(ones_row, 1.0)
            eps_ap = const.tile([1, 1], fp32)
            nc.vector.memset(eps_ap, EPS)

            for mi in range(NMACRO):
                c0, c1 = mi * NB, (mi + 1) * NB

                a16big = act.tile([128, 10, NB], bf16, name="a16big", bufs=2)
                aview = aT[:, c0:c1].rearrange("(f p) n -> p f n", p=128)
                nc.sync.dma_start(out=a16big[:, 0:5, :], in_=aview[:, 0:5, :])
                if mi == 0:
                    nc.sync.dma_start(out=w12big[:, 5:10, :], in_=w12v[:, 5:10, :])
                nc.sync.dma_start(out=a16big[:, 5:10, :], in_=aview[:, 5:10, :])
                rhs1 = [a16big[:, k, :] for k in range(10)]
                s16 = rhs1[2:]
                if mi == 0:
                    # w34 only feeds the final GEMM; keep it behind the
                    # first input tiles in the load FIFO
                    nc.sync.dma_start(
                        out=w34big, in_=w34[:, :].rearrange("(k p) m -> p k m", p=128)
                    )

                # ---- GEMM12: preT = (W1@W2)^T @ [x; s]  (host-folded) ----
                pre_a, pre_b = [], []
                for m in range(4):
                    pps = psum.tile([128, NB], fp32, name="mmps", tag="mm")
                    for nh in range(NH):
                        sl = slice(512 * nh, 512 * (nh + 1))
                        for k in range(10):
                            nc.tensor.matmul(
                                pps[:, sl],
                                w12t[k][:, 128 * m : 128 * (m + 1)],
                                rhs1[k][:, sl],
                                start=(k == 0),
                                stop=(k == 9),
                            )
                    ta = act.tile([128, NB], bf16, name=f"prea_{m}", bufs=2)
                    nc.scalar.activation(ta, pps, AF.Identity, bias=ppc(m, 12), scale=ppc(m, 11))
                    pre_a.append(ta)
                    if dual_pre:
                        tb = act.tile([128, NB], bf16, name=f"preb_{m}")
                        nc.scalar.activation(tb, pps, AF.Identity, bias=ppc(m, 14), scale=ppc(m, 13))
                        pre_b.append(tb)
                if not dual_pre:
                    pre_b = pre_a

                # ---- sigmoids ----
                sig0 = []
                for f in range(8):
                    t = act.tile([128, NB], bf16, name=f"sig0_{f}", bufs=2)
                    nc.scalar.activation(t, s16[f], AF.Sigmoid, bias=ppc(f, 5), scale=ppc(f, 4))
                    sig0.append(t)
                if share_sigma:
                    sig1 = [sig0[(f + 4) % 8] for f in range(8)]
                else:
                    sig1 = []
                    for f in range(8):
                        t = act.tile([128, NB], bf16, name=f"sig1_{f}")
                        nc.scalar.activation(
                            t, s16[(f + 4) % 8], AF.Sigmoid, bias=ppc(f, 7), scale=ppc(f, 6)
                        )
                        sig1.append(t)

                # ---- elementwise chain -> x16; LN stat accumulation on PE ----
                sxp = psum.tile([1, NB], fp32, name="statps", tag="stats")
                sqp = psum.tile([1, NB], fp32, name="statps", tag="stats")
                x16 = []
                for f in range(8):
                    # w0 = mc0/C*(E-s) = s*(-mc0/C) + mc0*E/C   [TS, 4x mode]
                    at = tmp.tile([128, NB], bf16, name="at", tag="at")
                    nc.gpsimd.tensor_scalar(
                        at, s16[f], ppc(f, 2), ppc(f, 0), OP.mult, OP.add
                    )
                    t0 = tmp.tile([128, NB], bf16, name="t0", tag="t0")
                    nc.vector.tensor_mul(out=t0, in0=at, in1=sig0[f])
                    bt = tmp.tile([128, NB], bf16, name="bt", tag="bt")
                    nc.gpsimd.tensor_scalar(
                        bt, s16[f], ppc(f, 3), ppc(f, 1), OP.mult, OP.add
                    )
                    t1 = tmp.tile([128, NB], bf16, name="t1", tag="t1")
                    nc.vector.tensor_mul(out=t1, in0=bt, in1=sig1[f])
                    u = tmp.tile([128, NB], bf16, name="u", tag="u")
                    nc.vector.tensor_add(out=u, in0=t0, in1=t1)
                    e2 = tmp.tile([128, NB], bf16, name="e2", tag="e2")
                    nc.vector.tensor_add(out=e2, in0=u, in1=s16[f])
                    xt = act.tile([128, NB], bf16, name=f"x16_{f}")
                    if f < 4:
                        nc.vector.tensor_add(out=xt, in0=e2, in1=pre_a[f])
                    else:
                        nc.vector.tensor_sub(out=xt, in0=e2, in1=pre_b[f - 4])
                    x16.append(xt)
                    sq = tmp.tile([128, NB], bf16, name="sq", tag="sq")
                    nc.scalar.activation(sq, xt, AF.Square, bias=0.0, scale=1.0)
                    for nh in range(NH):
                        sl = slice(512 * nh, 512 * (nh + 1))
                        nc.tensor.matmul(
                            sxp[:, sl], ones_col, xt[:, sl],
                            start=(f == 0), stop=(f == 7),
                        )
                        nc.tensor.matmul(
                            sqp[:, sl], ones_col, sq[:, sl],
                            start=(f == 0), stop=(f == 7),
                        )

                # ---- LN stats (sxp = mean, sqp = E[x^2] via 1/S ones) ----
                mu2 = small.tile([1, NB], fp32, name="mu2")
                nc.scalar.activation(mu2, sxp, AF.Square, bias=0.0, scale=1.0)
                var = small.tile([1, NB], fp32, name="var")
                nc.vector.tensor_sub(out=var, in0=sqp, in1=mu2)
                muRow = small.tile([1, NB], fp32, name="muRow")
                nc.scalar.activation(muRow, sxp, AF.Identity, bias=0.0, scale=1.0)
                rstd = small.tile([1, NB], fp32, name="rstd")
                if use_recip_fast:
                    # 1/sqrt(|var + eps|) in one ACT LUT op
                    nc.scalar.activation(
                        rstd, var, AF.Abs_reciprocal_sqrt, bias=eps_ap, scale=1.0
                    )
                else:
                    sd = small.tile([1, NB], fp32, name="sd")
                    nc.scalar.activation(sd, var, AF.Sqrt, bias=eps_ap, scale=1.0)
                    nc.vector.reciprocal(out=rstd, in_=sd)


                # ---- broadcast rows across partitions (K=1 matmuls) ----
                rbp = psum.tile([128, NB], fp32, name="mmps", tag="mm")
                mbp = psum.tile([128, NB], fp32, name="mmps", tag="mm")
                for nh in range(NH):
                    sl = slice(512 * nh, 512 * (nh + 1))
                    nc.tensor.matmul(rbp[:, sl], ones_row, rstd[:, sl], start=True, stop=True)
                    nc.tensor.matmul(mbp[:, sl], ones_row, muRow[:, sl], start=True, stop=True)
                rb16 = small.tile([128, NB], bf16, name="rb16")
                nc.scalar.activation(rb16, rbp, AF.Identity, bias=0.0, scale=1.0)
                mb16 = small.tile([128, NB], bf16, name="mb16")
                nc.scalar.activation(mb16, mbp, AF.Identity, bias=0.0, scale=1.0)

                # ---- normalize: y = (x*rstd - mu*rstd [*gamma]) + beta ----
                yo16big = act.tile([128, 10, NB], bf16, name="yo16big")
                y16 = []
                for f in range(8):
                    zt = tmp.tile([128, NB], bf16, name="zt", tag="zt")
                    nc.vector.tensor_sub(out=zt, in0=x16[f], in1=mb16)
                    yt = yo16big[:, f, :]
                    if gamma_trivial and beta_trivial:
                        nc.vector.tensor_mul(out=yt, in0=zt, in1=rb16)
                    else:
                        z2 = tmp.tile([128, NB], bf16, name="z2", tag="zt")
                        nc.vector.tensor_mul(out=z2, in0=zt, in1=rb16)
                        nc.vector.tensor_scalar(
                            yt, z2, ppc(f, 9), ppc(f, 8), OP.mult, OP.add
                        )
                    y16.append(yt)

                zview = zT[:S, c0:c1].rearrange("(f p) n -> p f n", p=128)
                nc.scalar.dma_start(out=zview[:, 0:4, :], in_=yo16big[:, 0:4, :])
                nc.scalar.dma_start(out=zview[:, 4:8, :], in_=yo16big[:, 4:8, :])

                # ---- GEMM3': out = W34^T @ y[:512]  (host-folded W3@W4) ----
                for m in range(2):
                    op4 = psum.tile([128, NB], fp32, name="mmps", tag="mm")
                    for nh in range(NH):
                        sl = slice(512 * nh, 512 * (nh + 1))
                        for k in range(4):
                            nc.tensor.matmul(
                                op4[:, sl],
                                w34t[k][:, 128 * m : 128 * (m + 1)],
                                y16[k][:, sl],
                                start=(k == 0),
                                stop=(k == 3),
                            )
                    nc.scalar.activation(
                        yo16big[:, 8 + m, :], op4, AF.Identity, bias=ppc(m, 15), scale=1.0
                    )
                nc.scalar.dma_start(
                    out=zT[S:, c0:c1].rearrange("(f p) n -> p f n", p=128),
                    in_=yo16big[:, 8:10, :],
                )

    _legalize_waits(nc)
    return nc


_CACHE = {}


def _get_nc(flags):
    if flags not in _CACHE:
        _CACHE[flags] = _build_nc(flags)
    return _CACHE[flags]


def kernel(inputs, states, W1, b1, W2, b2, W3, b3, W4, b4,
           excitatory_potential, inhibitory_potential, capacitance,
           max_conductance, mean_conductance_potential, std_conductance,
           gamma, beta, _trace=False):
    from concourse.bass_utils import run_bass_kernel_spmd

    f32 = np.float32
    inputs = np.asarray(inputs, f32)
    states = np.asarray(states, f32)

    # ---- host-side parameter folding (pi space) ----
    C = np.asarray(capacitance, f32)
    mc = np.asarray(max_conductance, f32)
    mean = np.asarray(mean_conductance_potential, f32)
    std = np.asarray(std_conductance, f32)
    E = np.asarray(excitatory_potential, f32)
    I = np.asarray(inhibitory_potential, f32)
    gam = np.asarray(gamma, f32)
    bet = np.asarray(beta, f32)

    Ep, Ip, Cp = E[PERM], I[PERM], C[PERM]
    mc0, mc1 = mc[PERM, 0], mc[PERM, 1]
    a0 = std[PERM, 0]
    cc0 = -std[PERM, 0] * mean[PERM, 0]
    a1 = std[PERM, 1]
    cc1 = -std[PERM, 1] * mean[PERM, 1]
    nmc0 = -mc0 / Cp
    nmc1 = -mc1 / Cp
    gp, bp = gam[PERM], bet[PERM]

    part = (PERM + S // 2) % S  # pi-index of partner's pi-position? see below
    # partner in pi-space of pi-feature j' is j'+-512; its own-params live at
    # pi-position (j'+512)%1024. share if a1/c1 match partner's a0/c0.
    pidx = np.concatenate([np.arange(S // 2, S), np.arange(0, S // 2)])
    share_sigma = bool(
        np.array_equal(a1, a0[pidx]) and np.array_equal(cc1, cc0[pidx])
    )
    C_ev, C_od = Cp[: S // 2], Cp[S // 2 :]
    b2a = np.asarray(b2, f32)
    dual_pre = not (np.array_equal(C_ev, C_od))
    gamma_trivial = bool(np.all(gp == 1.0))
    beta_trivial = bool(np.all(bp == 0.0))

    b12 = np.asarray(b1, f32) @ np.asarray(W2, f32) + b2a
    invCe = 1.0 / C_ev
    invCo = 1.0 / C_od
    b2e = b12 * invCe
    b2o = b12 * invCo

    def cols(v, n=8):
        # [n*128] vector -> per-partition [128, n] columns
        return np.ascontiguousarray(v.reshape(n, 128).T)

    pp = np.zeros((128, 8 * PPW), f32)
    mc0E = mc0 * Ep / Cp
    mc1I = mc1 * Ip / Cp
    for k, v in [(0, mc0E), (1, mc1I), (2, nmc0), (3, nmc1), (4, a0), (5, cc0),
                 (6, a1), (7, cc1), (8, bp), (9, gp), (10, np.asarray(b1, f32))]:
        pp[:, k::PPW] = cols(v)
    for k, v in [(11, invCe), (12, b2e), (13, invCo), (14, b2o)]:
        pp[:, k:k + 4 * PPW:PPW] = cols(v, 4)
    b34 = np.asarray(b3, f32) @ np.asarray(W4, f32) + np.asarray(b4, f32)
    pp[:, 15:15 + 2 * PPW:PPW] = cols(b34, 2)

    # ---- weights (bf16, pi-permuted W1 rows) ----
    W1p = np.asarray(W1, f32).copy()
    W1p[D:, :] = W1p[D + PERM, :]
    w12b = (W1p @ np.asarray(W2, f32)).astype(BF16)
    w34b = (np.asarray(W3, f32) @ np.asarray(W4, f32)).astype(BF16)

    # ---- activations: transpose + permute + bf16 ----
    aTb = np.ascontiguousarray(
        np.concatenate([inputs, states[:, PERM]], axis=1).T.astype(BF16)
    )  # [D+S, B]

    flags = (share_sigma, dual_pre, gamma_trivial, beta_trivial, False)
    nc = _get_nc(flags)

    in_maps = []
    for c in range(NCORES):
        sl = slice(c * BC, (c + 1) * BC)
        in_maps.append({
            "aT": np.ascontiguousarray(aTb[:, sl]),
            "w12": w12b, "w34": w34b, "pp": pp,
        })

    res = run_bass_kernel_spmd(nc, in_maps, core_ids=list(range(NCORES)),
                               trace=_trace)
    kernel._last = res

    zT = np.concatenate([np.asarray(r["zT"]) for r in res.results], axis=1)
    y_pi = zT[:S].astype(f32).T                 # [B, S] in pi order
    next_states = np.empty((B, S), f32)
    next_states[:, PERM] = y_pi
    out = np.ascontiguousarray(zT[S : S + O].astype(f32).T)  # [B, O]
    return (out, next_states)
